# revision 49
# baseline (speedup 1.0000x reference)
"""Multi-head attention (B=1, L=2048, D=1024, H=16) on 8 TRN2 NeuronCores.

Sharding: tensor-parallel over heads. Core i computes heads 2i, 2i+1:
  - projections with column shards of w_q/w_k/w_v (128 cols each)
  - full attention for its 2 heads
  - partial output projection with the matching 128-row shard of w_o
Host sums the 8 partial outputs and adds b_o once.

Fully software-pipelined single pass. ScalarE exp is the per-core floor
(2*2048^2 elems at 1 elem/cycle/lane ~= 74us); everything is arranged so
the exp stream never stalls:
  - st/exp stream ordered to FOLLOW INPUT-DMA ARRIVAL: quarters 0/1
    interleaved by k-quad (k quads land one at a time off the single
    HWDGE ring), then quarters 2/3 (all data resident)
  - av + denominator stream DECOUPLED from the exp stream: pt tiles park
    in SBUF and a greedy scheduler issues av/dc (PSUM-accumulated
    ones-matmuls; no VectorE reduction work) as pts and the per-quarter
    PSUM bank become available
  - k/v projections in kt-pair granularity, just-in-time; q by quarter
  - vh (natural [k,dh] layout) via PE transposes (DMA rings stay clear)
  - per-quarter tail: d copied out, partition-spread by DMA, one cheap
    reciprocal, K=1 fp32 broadcast matmul, normalize; split in two steps
    so the PE queue never waits on the spread round trip
  - output projection chases the last quarter's normalize per m-tile with
    ScalarE doing the final evacuations (it is idle after the last exp)
  - PE pre-warm + tail filler matmuls keep the HAM clock at 2.4 GHz
  - PSUM budget exactly 8 banks: st 2x2 + av 1 + dc 1 + shared mm 2
"""

import os
import numpy as np
import ml_dtypes

import concourse.bass as bass
import concourse.mybir as mybir
import concourse.tile as tile
from concourse import bacc
from concourse.bass import ts
from concourse.bass_utils import run_bass_kernel_spmd
from concourse.masks import make_identity

P = 128
L = 2048
D = 1024
DH = 64
NCORES = 8
NQ = 4  # q quarters
QW = 512  # quarter width
KT = 16  # k tiles of 128
TQ = 8  # contraction chunks of 128 for projections
BF16 = mybir.dt.bfloat16
F32 = mybir.dt.float32
AF = mybir.ActivationFunctionType
ALU = mybir.AluOpType

TRACE = False  # test.py flips this to get an NTFF profile / exec_time_ns
LAST_RESULT = {}

_CACHED_NC = None

# st/exp stream order: quarter pairs interleaved per k-quad (k/v quads land
# one at a time off the DMA ring for the first pair)
STREAM = []
for _qq in ((0, 1), (2, 3)):
    for _g in range(4):
        for _q in _qq:
            STREAM.extend((_q, 4 * _g + _j) for _j in range(4))
STREAM_POS = {e: i for i, e in enumerate(STREAM)}
AV_ORDER = [(q, kt) for q in range(NQ) for kt in range(KT)]


def _build():
    nc = bacc.Bacc("TRN2", target_bir_lowering=False, debug=False, num_devices=NCORES)

    qT = nc.dram_tensor("qT", [P, NQ, TQ, QW], BF16, kind="ExternalInput")
    kT = nc.dram_tensor("kT", [P, KT, TQ, P], BF16, kind="ExternalInput")
    vT = nc.dram_tensor("vT", [P, KT, TQ, P], BF16, kind="ExternalInput")
    wq = nc.dram_tensor("wq", [P, TQ, P], BF16, kind="ExternalInput")
    wk = nc.dram_tensor("wk", [P, TQ, P], BF16, kind="ExternalInput")
    wv = nc.dram_tensor("wv", [P, TQ, P], BF16, kind="ExternalInput")
    bq = nc.dram_tensor("bq", [P, 1], F32, kind="ExternalInput")
    bk = nc.dram_tensor("bk", [P, 1], F32, kind="ExternalInput")
    bv = nc.dram_tensor("bv", [P, 1], F32, kind="ExternalInput")
    wo = nc.dram_tensor("wo", [P, D], BF16, kind="ExternalInput")
    out = nc.dram_tensor("out", [KT, P, D], BF16, kind="ExternalOutput")

    with tile.TileContext(nc) as tc:
        with (
            tc.tile_pool(name="const", bufs=1) as const_pool,
            tc.tile_pool(name="inputs", bufs=1) as in_pool,
            tc.tile_pool(name="proj", bufs=1) as proj_pool,
            tc.tile_pool(name="work", bufs=1) as work_pool,
            tc.tile_pool(name="pt_pool", bufs=18) as pt_pool,
            tc.tile_pool(name="osb_pool", bufs=3) as osb_pool,
        ):
            ones_b = const_pool.tile([P, 1], BF16)
            nc.vector.memset(ones_b[:], 1.0)
            ones_b2 = const_pool.tile([1, DH], BF16)
            nc.vector.memset(ones_b2[:], 1.0)
            ones_f = const_pool.tile([1, DH], F32)
            nc.vector.memset(ones_f[:], 1.0)
            dummy_src = const_pool.tile([P, QW], BF16)
            nc.vector.memset(dummy_src[:], 0.5)
            identity = const_pool.tile([P, P], BF16)
            make_identity(nc, identity[:])
            warm = const_pool.tile([1, 32], F32)
            # preload the exp table set while input DMAs stream
            nc.scalar.activation(warm[:], ones_f[0:1, 0:32], AF.Exp)

            # ---- stage inputs ----
            wq_sb = in_pool.tile([P, TQ, P], BF16)
            wk_sb = in_pool.tile([P, TQ, P], BF16)
            wv_sb = in_pool.tile([P, TQ, P], BF16)
            bq_sb = in_pool.tile([P, 1], F32)
            bk_sb = in_pool.tile([P, 1], F32)
            bv_sb = in_pool.tile([P, 1], F32)
            wo_sb = in_pool.tile([P, D], BF16)
            qT_sb = in_pool.tile([P, NQ, TQ, QW], BF16)
            kT_sb = in_pool.tile([P, KT, TQ, P], BF16)
            vT_sb = in_pool.tile([P, KT, TQ, P], BF16)

            # Bulk input on the sync HWDGE ring, ordered by first consumption
            # in the st/exp stream (triggers block in-queue on ring depth, so
            # the scalar queue must stay clear or exps starve behind them).
            # Tiny weights ride the slow gpsimd/SWDGE.
            nc.gpsimd.dma_start(wk_sb[:], wk[:])
            nc.gpsimd.dma_start(bk_sb[:], bk[:])
            nc.gpsimd.dma_start(wv_sb[:], wv[:])
            nc.gpsimd.dma_start(bv_sb[:], bv[:])
            nc.gpsimd.dma_start(wo_sb[:], wo[:])
            nc.sync.dma_start(wq_sb[:], wq[:])
            nc.sync.dma_start(bq_sb[:], bq[:])
            nc.sync.dma_start(kT_sb[:, 0:2], kT[:, 0:2])
            nc.sync.dma_start(qT_sb[:, 0], qT[:, 0])
            nc.sync.dma_start(qT_sb[:, 1], qT[:, 1])
            nc.sync.dma_start(kT_sb[:, 2:4], kT[:, 2:4])
            nc.sync.dma_start(vT_sb[:, 0:2], vT[:, 0:2])
            nc.sync.dma_start(kT_sb[:, 4:8], kT[:, 4:8])
            nc.sync.dma_start(vT_sb[:, 2:4], vT[:, 2:4])
            nc.sync.dma_start(vT_sb[:, 4:8], vT[:, 4:8])
            nc.sync.dma_start(kT_sb[:, 8:12], kT[:, 8:12])
            nc.sync.dma_start(kT_sb[:, 12:16], kT[:, 12:16])
            nc.sync.dma_start(vT_sb[:, 8:12], vT[:, 8:12])
            nc.sync.dma_start(qT_sb[:, 2], qT[:, 2])
            nc.sync.dma_start(vT_sb[:, 12:16], vT[:, 12:16])
            nc.sync.dma_start(qT_sb[:, 3], qT[:, 3])

            # projection outputs (heads on partitions: h*64..h*64+63)
            qhT = proj_pool.tile([P, L], BF16)
            khT = proj_pool.tile([P, L], BF16)
            vhT = proj_pool.tile([P, L], BF16)
            vh = proj_pool.tile([P, KT, P], BF16)  # natural [k, dh] layout

            lhsT_c = work_pool.tile([P, L], BF16)  # normalized concat^T
            u_sb = work_pool.tile([P, QW], F32)
            dsb = work_pool.tile([1, 2 * QW], F32)  # d rows gathered
            dsp = work_pool.tile([DH, 2 * QW // DH], F32)  # spread for recip
            dspb = work_pool.tile([DH, 2 * QW // DH], BF16)  # 1/d spread
            dcr = work_pool.tile([1, 2 * QW], BF16)  # 1/d back in row layout
            dacc2 = work_pool.tile([P, 2 * QW], BF16)  # q2 denominator acc
            dacc3 = work_pool.tile([P, 2 * QW], BF16)  # q3 denominator acc

            with (
                tc.tile_pool(name="st_ps", bufs=2, space="PSUM") as st_ps,
                tc.tile_pool(name="av_ps", bufs=1, space="PSUM") as av_ps,
                tc.tile_pool(name="dc_ps", bufs=1, space="PSUM") as dc_ps,
                tc.tile_pool(name="mm_ps", bufs=2, space="PSUM") as mm_ps,
            ):

                def pe_warm(n, name):
                    ps = mm_ps.tile([P, QW], F32, tag="mm", name=name)
                    for _ in range(n):
                        nc.tensor.matmul(ps[:], dummy_src[:, 0:P], dummy_src[:])

                def qproj(qi):
                    ps = mm_ps.tile([P, QW], F32, tag="mm", name=f"qp{qi}")
                    for t in range(TQ):
                        nc.tensor.matmul(
                            ps[:],
                            wq_sb[:, t, :],
                            qT_sb[:, qi, t, :],
                            start=(t == 0),
                            stop=(t == TQ - 1),
                        )
                    nc.vector.tensor_scalar(
                        qhT[:, ts(qi, QW)], ps[:], bq_sb[:], None, op0=ALU.add
                    )

                def kproj(pp):  # kt pair 2pp, 2pp+1
                    ps = mm_ps.tile([P, QW], F32, tag="mm", name=f"kp{pp}")
                    for t in range(TQ):
                        nc.tensor.matmul(
                            ps[:, 0 : 2 * P],
                            wk_sb[:, t, :],
                            kT_sb[:, 2 * pp : 2 * pp + 2, t, :],
                            start=(t == 0),
                            stop=(t == TQ - 1),
                        )
                    nc.vector.tensor_scalar(
                        khT[:, 2 * pp * P : (2 * pp + 2) * P],
                        ps[:, 0 : 2 * P],
                        bk_sb[:],
                        None,
                        op0=ALU.add,
                    )

                def kproj_q(g):  # kt quad (full N=512 matmuls, LDW hidden)
                    ps = mm_ps.tile([P, QW], F32, tag="mm", name=f"kq{g}")
                    for t in range(TQ):
                        nc.tensor.matmul(
                            ps[:],
                            wk_sb[:, t, :],
                            kT_sb[:, ts(g, 4), t, :],
                            start=(t == 0),
                            stop=(t == TQ - 1),
                        )
                    nc.vector.tensor_scalar(
                        khT[:, ts(g, QW)], ps[:], bk_sb[:], None, op0=ALU.add
                    )

                def vproj(pp):  # kt pair + PE transposes (early, ring busy)
                    ps = mm_ps.tile([P, QW], F32, tag="mm", name=f"vp{pp}")
                    for t in range(TQ):
                        nc.tensor.matmul(
                            ps[:, 0 : 2 * P],
                            wv_sb[:, t, :],
                            vT_sb[:, 2 * pp : 2 * pp + 2, t, :],
                            start=(t == 0),
                            stop=(t == TQ - 1),
                        )
                    nc.vector.tensor_scalar(
                        vhT[:, 2 * pp * P : (2 * pp + 2) * P],
                        ps[:, 0 : 2 * P],
                        bv_sb[:],
                        None,
                        op0=ALU.add,
                    )
                    for kt in (2 * pp, 2 * pp + 1):
                        pst = mm_ps.tile([P, P], BF16, tag="mm", name=f"vt{kt}")
                        nc.tensor.transpose(pst[:], vhT[:, ts(kt, P)], identity[:])
                        nc.vector.tensor_copy(vh[:, kt, :], pst[:])

                def vproj_q(g):  # kt quad + xbar DMA transpose (ring idle)
                    ps = mm_ps.tile([P, QW], F32, tag="mm", name=f"vq{g}")
                    for t in range(TQ):
                        nc.tensor.matmul(
                            ps[:],
                            wv_sb[:, t, :],
                            vT_sb[:, ts(g, 4), t, :],
                            start=(t == 0),
                            stop=(t == TQ - 1),
                        )
                    nc.vector.tensor_scalar(
                        vhT[:, ts(g, QW)], ps[:], bv_sb[:], None, op0=ALU.add
                    )
                    nc.sync.dma_start_transpose(
                        vh[:, ts(g, 4), :], vhT[:, ts(g, QW)]
                    )

                pt_tiles = {}
                avdc_tiles = {}

                def do_st_exp(e):
                    qi, kt = e
                    st_t = st_ps.tile(
                        [P, 2 * QW], F32, tag="st", name=f"st{qi}_{kt}"
                    )
                    for h in (0, 1):
                        nc.tensor.matmul(
                            st_t[:, ts(h, QW)],
                            khT[ts(h, DH), ts(kt, P)],
                            qhT[ts(h, DH), ts(qi, QW)],
                        )
                    pt_t = pt_pool.tile(
                        [P, 2 * QW], BF16, tag="pt", name=f"pt{qi}_{kt}"
                    )
                    nc.scalar.activation(pt_t[:], st_t[:], AF.Exp, scale=0.125)
                    pt_tiles[e] = pt_t

                def do_av_dc(e):
                    qi, kt = e
                    if kt == 0:
                        av_t = av_ps.tile([P, QW], F32, tag="av", name=f"av{qi}")
                        dc_t = dc_ps.tile([33, QW], F32, tag="dc", name=f"dc{qi}")
                        avdc_tiles[qi] = (av_t, dc_t)
                    av_t, dc_t = avdc_tiles[qi]
                    pt_t = pt_tiles.pop(e)
                    first = kt == 0
                    last = kt == KT - 1
                    for h in (0, 1):
                        nc.tensor.matmul(
                            av_t[ts(h, DH), :],
                            vh[:, kt, ts(h, DH)],
                            pt_t[:, ts(h, QW)],
                            start=first,
                            stop=last,
                            tile_position=(0, DH * h),
                        )
                    if qi < 2:
                        # denominator via PE ones-matmuls (PSUM accumulated)
                        nc.tensor.matmul(
                            dc_t[0:1, :],
                            ones_b[:, 0:1],
                            pt_t[:, 0:QW],
                            start=first,
                            stop=last,
                            tile_position=(0, 0),
                        )
                        nc.tensor.matmul(
                            dc_t[32:33, :],
                            ones_b[:, 0:1],
                            pt_t[:, QW : 2 * QW],
                            start=first,
                            stop=last,
                            tile_position=(0, 32),
                        )
                    else:
                        # PE is the pacer by then: accumulate on VectorE,
                        # partition-reduce once at the end of the quarter
                        dacc = dacc2 if qi == 2 else dacc3
                        if first:
                            nc.vector.tensor_copy(dacc[:], pt_t[:])
                        else:
                            with nc.allow_low_precision(
                                reason="denominator partials; ~0.2% rounding "
                                "averages out over 16 adds, tolerance is 2e-2"
                            ):
                                nc.vector.tensor_tensor(
                                    dacc[:], dacc[:], pt_t[:], op=ALU.add
                                )
                        if last:
                            nc.tensor.matmul(
                                dc_t[0:1, :], ones_b[:, 0:1], dacc[:, 0:QW]
                            )
                            nc.tensor.matmul(
                                dc_t[32:33, :],
                                ones_b[:, 0:1],
                                dacc[:, QW : 2 * QW],
                                tile_position=(0, 32),
                            )

                def tail_a(qi):
                    # d -> spread -> 1/d; u evacuated. DVE reciprocal cost is
                    # ~6.5ns per free-dim element, hence the partition spread.
                    av_t, dc_t = avdc_tiles[qi]
                    nc.vector.tensor_copy(u_sb[:], av_t[:])
                    nc.vector.tensor_copy(dsb[0:1, 0:QW], dc_t[0:1, :])
                    nc.vector.tensor_copy(dsb[0:1, QW : 2 * QW], dc_t[32:33, :])
                    nc.sync.dma_start(dsp[:], dsb[:])
                    with nc.allow_low_precision(
                        reason="1/d in bf16: ~0.2% rounding, tolerance 2e-2"
                    ):
                        nc.vector.reciprocal(dspb[:], dsp[:])
                    nc.sync.dma_start(dcr[:], dspb[:])

                def tail_b(qi, last=False):
                    # broadcast 1/d over head partition groups, normalize
                    bc_t = mm_ps.tile([P, QW], F32, tag="mm", name=f"bc{qi}")
                    nc.tensor.matmul(
                        bc_t[0:DH, :],
                        ones_b2[0:1, :],
                        dcr[0:1, 0:QW],
                        tile_position=(0, 0),
                    )
                    nc.tensor.matmul(
                        bc_t[DH:P, :],
                        ones_b2[0:1, :],
                        dcr[0:1, QW : 2 * QW],
                        tile_position=(0, DH),
                    )
                    if last:
                        for j in range(4):
                            nc.vector.tensor_tensor(
                                lhsT_c[:, qi * QW + j * P : qi * QW + (j + 1) * P],
                                u_sb[:, ts(j, P)],
                                bc_t[:, ts(j, P)],
                                op=ALU.mult,
                            )
                    else:
                        nc.vector.tensor_tensor(
                            lhsT_c[:, ts(qi, QW)], u_sb[:], bc_t[:], op=ALU.mult
                        )

                def outproj(m, on_scalar=False):
                    osb = osb_pool.tile([P, D], BF16, tag="osb", name=f"osb{m}")
                    for n in (0, 1):
                        ps = mm_ps.tile([P, QW], F32, tag="mm", name=f"op{m}_{n}")
                        nc.tensor.matmul(
                            ps[:], lhsT_c[:, ts(m, P)], wo_sb[:, ts(n, QW)]
                        )
                        if on_scalar:
                            nc.scalar.copy(osb[:, ts(n, QW)], ps[:])
                        else:
                            nc.vector.tensor_copy(osb[:, ts(n, QW)], ps[:])
                    nc.sync.dma_start(out[m], osb[:])

                # ---- PE pre-warm while the first input chunks stream ----
                pe_warm(12, "warm0")

                # ---- head: minimum to start the exp stream ----
                kproj(0)
                qproj(0)

                # Just-in-time projection fillers keyed by stream position.
                # Producers MUST be issued before their consumers (Tile only
                # sees hazards against already-issued writers); k/q pieces sit
                # right before the st that reads them, v pieces are placed at
                # their data's expected DMA-arrival position (the decoupled
                # av stream is gated on them explicitly via vp_done).
                vp_done = 0

                def vproj_c(pp):
                    nonlocal vp_done
                    vproj(pp)
                    vp_done += 1

                def vproj_qc(g):
                    nonlocal vp_done
                    vproj_q(g)
                    vp_done += 2

                fillers = {
                    1: [lambda: kproj(1)],
                    3: [lambda: qproj(1)],
                    5: [lambda: vproj_c(0)],
                    7: [lambda: kproj_q(1)],
                    10: [lambda: vproj_c(1)],
                    13: [lambda: vproj_qc(1)],
                    15: [lambda: kproj_q(2)],
                    20: [lambda: kproj_q(3)],
                    23: [lambda: vproj_qc(2)],
                    29: [lambda: vproj_qc(3)],
                    30: [lambda: qproj(2)],
                    34: [lambda: qproj(3)],
                }

                from collections import defaultdict

                pending = defaultdict(list)
                av_ptr = 0
                P_TOT = 64 + 24
                for p in range(P_TOT):
                    if p < 64:
                        do_st_exp(STREAM[p])
                    for fn in fillers.get(p, []):
                        fn()
                    for fn in pending.pop(p, []):
                        fn()
                    n = 0
                    cap = 2 + (p >= 48) + (p >= 62)
                    while av_ptr < len(AV_ORDER) and n < cap:
                        e = AV_ORDER[av_ptr]
                        if STREAM_POS[e] >= p or e[1] >= 2 * vp_done:
                            break
                        do_av_dc(e)
                        av_ptr += 1
                        n += 1
                        if e[1] == KT - 1:
                            qq = e[0]
                            is_last = qq == NQ - 1
                            tail_a(qq)
                            if is_last:
                                # keep the PE warm through the recip round trip
                                pe_warm(6, "warmt")
                            pending[p + 2].append(
                                lambda qq=qq, il=is_last: tail_b(qq, last=il)
                            )
                            for j in range(4):
                                pending[p + 3 + j].append(
                                    lambda m=4 * qq + j, il=is_last: outproj(
                                        m, on_scalar=il
                                    )
                                )
                            break  # don't start the next quarter's av this pos
                assert av_ptr == len(AV_ORDER), av_ptr
                for p2 in sorted(pending):
                    for fn in pending[p2]:
                        fn()

    nc.compile()
    return nc


def kernel(q, k, v, w_q, b_q, w_k, b_k, w_v, b_v, w_o, b_o):
    global _CACHED_NC, LAST_RESULT
    if _CACHED_NC is None:
        _CACHED_NC = _build()
    nc = _CACHED_NC

    bf16 = ml_dtypes.bfloat16

    qTf = np.ascontiguousarray(np.asarray(q, np.float32)[0].T)  # [D, L]
    kTf = np.ascontiguousarray(np.asarray(k, np.float32)[0].T)
    vTf = np.ascontiguousarray(np.asarray(v, np.float32)[0].T)
    q2 = np.ascontiguousarray(
        qTf.reshape(TQ, P, NQ, QW).transpose(1, 2, 0, 3)
    ).astype(bf16)
    k2 = np.ascontiguousarray(
        kTf.reshape(TQ, P, KT, P).transpose(1, 2, 0, 3)
    ).astype(bf16)
    v2 = np.ascontiguousarray(
        vTf.reshape(TQ, P, KT, P).transpose(1, 2, 0, 3)
    ).astype(bf16)

    w_q = np.asarray(w_q, np.float32)
    w_k = np.asarray(w_k, np.float32)
    w_v = np.asarray(w_v, np.float32)
    w_o = np.asarray(w_o, np.float32)
    b_q = np.asarray(b_q, np.float32)
    b_k = np.asarray(b_k, np.float32)
    b_v = np.asarray(b_v, np.float32)
    b_o = np.asarray(b_o, np.float32)

    def tile_w(w):  # [D, 128] -> [128, D//128, 128] contiguous
        return np.ascontiguousarray(
            w.reshape(TQ, P, P).transpose(1, 0, 2)
        ).astype(bf16)

    in_maps = []
    for i in range(NCORES):
        sl = slice(P * i, P * (i + 1))
        in_maps.append(
            {
                "qT": q2,
                "kT": k2,
                "vT": v2,
                "wq": tile_w(w_q[:, sl]),
                "wk": tile_w(w_k[:, sl]),
                "wv": tile_w(w_v[:, sl]),
                "bq": np.ascontiguousarray(b_q[sl]).reshape(P, 1),
                "bk": np.ascontiguousarray(b_k[sl]).reshape(P, 1),
                "bv": np.ascontiguousarray(b_v[sl]).reshape(P, 1),
                "wo": np.ascontiguousarray(w_o[sl, :]).astype(bf16),
            }
        )

    kwargs = {}
    if TRACE:
        import shutil

        tdir = "/tmp/bass_trace"
        shutil.rmtree(tdir, ignore_errors=True)
        os.makedirs(tdir, exist_ok=True)
        kwargs["tmpdir"] = tdir
    res = run_bass_kernel_spmd(nc, in_maps, list(range(NCORES)), trace=TRACE, **kwargs)
    LAST_RESULT = {
        "exec_time_ns": res.exec_time_ns,
        "trace_path": (res.instructions_and_trace or (None, None))[1],
    }
    acc = np.zeros((L, D), np.float64)
    for i in range(NCORES):
        acc += res.results[i]["out"].reshape(L, D).astype(np.float64)
    acc += b_o.astype(np.float64)
    return acc.astype(np.float32).reshape(1, L, D)
